# revision 9
# baseline (speedup 1.0000x reference)
"""MoE (top-2 of 8 experts, SwiGLU) kernel for 8 TRN2 NeuronCores.

Expert-parallel sparse strategy. Core e receives only the tokens routed to
expert e (host-side all-to-all dispatch, grouped by token-range chunk and
padded to a per-chunk uniform capacity so all 8 cores run one SPMD program):

  phase 1: aT = silu(hs@Wg)*(hs@Wu) for the compact token set (bf16 matmuls,
           fp32 PSUM), blocks of <=512 tokens.
  phase 2: y = (aT @ Wd) * combine_weight, token-major 128-row tiles,
           indirect-scattered into per-chunk DRAM buffers at chunk-local
           token rows (pads land on a trash row).
  combine: per-chunk ReduceScatter(add) across the 8 cores, issued inline
           right after the chunk's last scatter so it overlaps the next
           chunk's compute; the RS writes each core's token shard directly.

DMA layouts are host-pre-tiled so every device DMA is contiguous per
partition (the naive [H, C] column-slice loads cost ~114us in 1KB lines).

Matmul operands are bf16 (fp32 accumulation): rel err vs the fp32 reference
~5e-3, well inside the 2e-2 gate.
"""

import numpy as np
import ml_dtypes

import jax
import concourse.bass as bass
import concourse.tile as tile
from concourse import bacc, mybir
from concourse.bass import ts

E, H, I, T, KTOP = 8, 2048, 1408, 4096, 2
NC = 8
HC, IC = H // 128, I // 128
BF16 = mybir.dt.bfloat16
F32 = mybir.dt.float32

CHUNK_BOUNDS = (2560, 4096)


def _blocks_of(C, caps=None):
    """Phase-1 blocks of <=512 slots. When `caps` is given, blocks never
    cross a chunk-segment boundary: the segment's last compute block is then
    small, so the chunk's final scatter (and its ReduceScatter) issues with
    the whole next segment's compute still pending to hide behind."""
    if caps is None:
        caps = [C]
    blocks, pos = [], 0
    for seg_end in np.cumsum(caps):
        seg_end = int(seg_end)
        while pos < seg_end:
            nb = min(512, seg_end - pos)
            blocks.append((int(pos), int(nb)))
            pos += nb
    return blocks


def build_kernel(chunk_lens, caps, rs_to_out=False, wsplit=True,
                 silu_fused=True, use_rs=True, defer_zero=True):
    """One SPMD program. chunk_lens[j]: token count of chunk j (mult of NC);
    caps[j]: compact-slot capacity per chunk (mult of 128). Per-core output:
    [T//NC, H] bf16, rows grouped by chunk."""
    nchunks = len(chunk_lens)
    assert sum(chunk_lens) == T
    C = sum(caps)
    assert all(c % 128 == 0 for c in caps)
    assert all(l % NC == 0 for l in chunk_lens)

    nc = bacc.Bacc("TRN2", target_bir_lowering=False, debug=False,
                   num_devices=NC)
    blocks = _blocks_of(C, caps)
    hs_el = sum(HC * nb for _, nb in blocks)
    hsTiled = nc.declare_dram_parameter("hsTiled", [128, hs_el], BF16,
                                        isOutput=False).ap()
    wgT = nc.declare_dram_parameter("wgT", [128, HC * I], BF16, isOutput=False).ap()
    wuT = nc.declare_dram_parameter("wuT", [128, HC * I], BF16, isOutput=False).ap()
    wdT = nc.declare_dram_parameter("wdT", [128, IC * H], BF16, isOutput=False).ap()
    wcg = nc.declare_dram_parameter("wcg", [C], F32, isOutput=False).ap()
    sidx = nc.declare_dram_parameter("sidx", [C], mybir.dt.int32,
                                     isOutput=False).ap()
    out = nc.declare_dram_parameter("out", [T // NC, H], BF16,
                                    isOutput=True).ap()

    silu = mybir.ActivationFunctionType.Silu
    rgroups = [list(range(NC))]
    nct = C // 128
    seg_lo = [sum(caps[:j]) for j in range(nchunks)]
    out_lo = [sum(chunk_lens[:j]) // NC for j in range(nchunks)]

    with tile.TileContext(nc) as tc:
        with (
            tc.tile_pool(name="wpool", bufs=1) as wpool,
            tc.tile_pool(name="hspool", bufs=2) as hspool,
            tc.tile_pool(name="apool", bufs=1) as apool,
            tc.tile_pool(name="stage", bufs=3) as stage,
            tc.tile_pool(name="ypool", bufs=3) as ypool,
            tc.tile_pool(name="pg", bufs=2, space="PSUM") as pg,
            tc.tile_pool(name="pu", bufs=2, space="PSUM") as pu,
            tc.tile_pool(name="py", bufs=2, space="PSUM") as py,
            tc.tile_pool(name="dram", bufs=1, space="DRAM") as dram,
        ):
            wg_sb = wpool.tile([128, HC, I], BF16, tag="wg")
            wu_sb = wpool.tile([128, HC, I], BF16, tag="wu")
            wd_sb = wpool.tile([128, IC, H], BF16, tag="wd")
            wgv = wgT.rearrange("p (c i) -> p c i", c=HC)
            wuv = wuT.rearrange("p (c i) -> p c i", c=HC)
            wdv = wdT.rearrange("p (c j) -> p c j", c=IC)
            if wsplit:
                for c in range(HC):
                    nc.sync.dma_start(out=wg_sb[:, c, :], in_=wgv[:, c, :])
                    nc.sync.dma_start(out=wu_sb[:, c, :], in_=wuv[:, c, :])
                for c in range(IC):
                    nc.sync.dma_start(out=wd_sb[:, c, :], in_=wdv[:, c, :])
            else:
                nc.sync.dma_start(out=wg_sb[:], in_=wgv)
                nc.sync.dma_start(out=wu_sb[:], in_=wuv)
                nc.sync.dma_start(out=wd_sb[:], in_=wdv)
            wcg_sb = wpool.tile([128, nct], F32, tag="wcg")
            nc.sync.dma_start(out=wcg_sb[:], in_=wcg.rearrange("(ct p) -> p ct", p=128))
            sidx_sb = wpool.tile([128, nct], mybir.dt.int32, tag="sidx")
            nc.sync.dma_start(out=sidx_sb[:], in_=sidx.rearrange("(ct p) -> p ct", p=128))
            zsb = wpool.tile([128, H], BF16, tag="zero")
            nc.vector.memset(zsb[:], 0.0)

            pts, rss = [], []
            for j in range(nchunks):
                pt = dram.tile([chunk_lens[j] + 128, H], BF16,
                               name=f"pt{j}", tag=f"pt{j}")
                pts.append(pt)
                if not defer_zero or j == 0:
                    for q in range(chunk_lens[j] // 128):
                        nc.sync.dma_start(out=pt[ts(q, 128), :], in_=zsb[:])
                rss.append(dram.tile([chunk_lens[j] // NC, H], BF16,
                                     name=f"rs{j}", tag=f"rs{j}"))

            def finish_chunk(j):
                sh = chunk_lens[j] // NC
                if use_rs and rs_to_out:
                    nc.gpsimd.collective_compute(
                        "ReduceScatter", mybir.AluOpType.add,
                        replica_groups=rgroups,
                        ins=[pts[j][:chunk_lens[j], :].opt()],
                        outs=[out[out_lo[j]:out_lo[j] + sh, :].opt()])
                elif use_rs:
                    nc.gpsimd.collective_compute(
                        "ReduceScatter", mybir.AluOpType.add,
                        replica_groups=rgroups,
                        ins=[pts[j][:chunk_lens[j], :].opt()],
                        outs=[rss[j][:].opt()])
                    nc.sync.dma_start(out=out[out_lo[j]:out_lo[j] + sh, :],
                                      in_=rss[j][:])
                else:
                    nc.sync.dma_start(out=out[out_lo[j]:out_lo[j] + sh, :],
                                      in_=pts[j][:sh, :])

            hs_off = 0
            zeros_pending = defer_zero
            for (pos, nb) in blocks:
                hs_t = hspool.tile([128, HC, nb], BF16, tag="hst")
                nc.sync.dma_start(
                    out=hs_t[:],
                    in_=hsTiled[:, hs_off:hs_off + HC * nb]
                        .rearrange("p (c t) -> p c t", c=HC))
                hs_off += HC * nb

                if zeros_pending:
                    # deferred pt zeroing: issued after block0's hs load so
                    # these 8+MB of writes stay off the startup DMA window
                    zeros_pending = False
                    for j in range(1, nchunks):
                        for q in range(chunk_lens[j] // 128):
                            nc.sync.dma_start(out=pts[j][ts(q, 128), :],
                                              in_=zsb[:])

                aT = apool.tile([128, IC, nb], BF16, tag="aT")
                for it in range(IC):
                    psg = pg.tile([128, nb], F32, tag="psg")
                    psu = pu.tile([128, nb], F32, tag="psu")
                    for c in range(HC):
                        nc.tensor.matmul(psg[:], lhsT=wg_sb[:, c, ts(it, 128)],
                                         rhs=hs_t[:, c, :],
                                         start=(c == 0), stop=(c == HC - 1))
                    for c in range(HC):
                        nc.tensor.matmul(psu[:], lhsT=wu_sb[:, c, ts(it, 128)],
                                         rhs=hs_t[:, c, :],
                                         start=(c == 0), stop=(c == HC - 1))
                    sil = stage.tile([128, nb], F32, tag="sil")
                    if silu_fused:
                        nc.scalar.activation(out=sil[:], in_=psg[:], func=silu)
                        nc.vector.tensor_mul(aT[:, it, :], sil[:], psu[:])
                    else:
                        nc.scalar.activation(
                            out=sil[:], in_=psg[:],
                            func=mybir.ActivationFunctionType.Sigmoid)
                        nc.vector.tensor_mul(sil[:], sil[:], psg[:])
                        nc.vector.tensor_mul(aT[:, it, :], sil[:], psu[:])

                for ct in range(nb // 128):
                    gct = pos // 128 + ct
                    row = gct * 128
                    j = next(jj for jj in range(nchunks)
                             if seg_lo[jj] <= row < seg_lo[jj] + caps[jj])
                    y_sb = ypool.tile([128, H], BF16, tag="ysb")
                    for hb in range(H // 512):
                        psy = py.tile([128, 512], F32, tag="psy")
                        for c2 in range(IC):
                            nc.tensor.matmul(psy[:],
                                             lhsT=aT[:, c2, ts(ct, 128)],
                                             rhs=wd_sb[:, c2, ts(hb, 512)],
                                             start=(c2 == 0), stop=(c2 == IC - 1))
                        nc.vector.tensor_scalar_mul(
                            y_sb[:, ts(hb, 512)], psy[:],
                            wcg_sb[:, gct:gct + 1])
                    nc.gpsimd.indirect_dma_start(
                        out=pts[j][:],
                        out_offset=bass.IndirectOffsetOnAxis(
                            ap=sidx_sb[:, gct:gct + 1], axis=0),
                        in_=y_sb[:],
                        in_offset=None)
                    if row + 128 == seg_lo[j] + caps[j]:
                        finish_chunk(j)

    nc.compile()
    return nc


class _Runner:
    """Compile once, execute many; device-resident staged inputs."""

    def __init__(self, nc, n_cores):
        from concourse import bass2jax
        from jax.experimental.shard_map import shard_map
        from jax.sharding import Mesh, PartitionSpec

        bass2jax.install_neuronx_cc_hook()
        partition_name = (nc.partition_id_tensor.name
                          if nc.partition_id_tensor else None)

        in_names, out_names, out_avals, zero_outs = [], [], [], []
        for alloc in nc.m.functions[0].allocations:
            if not isinstance(alloc, mybir.MemoryLocationSet):
                continue
            name = alloc.memorylocations[0].name
            if alloc.kind == "ExternalInput":
                if name != partition_name:
                    in_names.append(name)
            elif alloc.kind == "ExternalOutput":
                shape = tuple(alloc.tensor_shape)
                dtype = mybir.dt.np(alloc.dtype)
                out_names.append(name)
                out_avals.append(jax.core.ShapedArray(shape, dtype))
                zero_outs.append(np.zeros(shape, dtype))
        self.n_params = len(in_names)
        self.param_names = list(in_names)
        self.out_names = out_names
        self.out_avals = out_avals
        self.n_cores = n_cores
        all_names = in_names + out_names
        if partition_name is not None:
            all_names.append(partition_name)

        def _body(*args):
            operands = list(args)
            if partition_name is not None:
                operands.append(bass2jax.partition_id_tensor())
            outs = bass2jax._bass_exec_p.bind(
                *operands,
                out_avals=tuple(out_avals),
                in_names=tuple(all_names),
                out_names=tuple(out_names),
                lowering_input_output_aliases=(),
                sim_require_finite=True,
                sim_require_nnan=True,
                nc=nc,
            )
            return tuple(outs)

        devices = jax.devices()[:n_cores]
        assert len(devices) == n_cores
        mesh = Mesh(np.asarray(devices), ("core",))
        n_ops = self.n_params + len(out_names)
        self._mesh = mesh
        self._fn = jax.jit(
            shard_map(_body, mesh=mesh,
                      in_specs=(PartitionSpec("core"),) * n_ops,
                      out_specs=(PartitionSpec("core"),) * len(out_names),
                      check_rep=False),
            keep_unused=True)
        self._zeros = [
            np.zeros((n_cores * z.shape[0], *z.shape[1:]), z.dtype)
            for z in zero_outs
        ]
        self._dev_args = None

    def prepare(self, in_maps):
        from jax.sharding import NamedSharding, PartitionSpec
        sh = NamedSharding(self._mesh, PartitionSpec("core"))
        concat = [
            np.concatenate([np.asarray(in_maps[c][name])
                            for c in range(self.n_cores)], axis=0)
            for name in self.param_names
        ]
        self._dev_args = [jax.device_put(a, sh) for a in concat + self._zeros]

    def execute(self):
        outs = self._fn(*self._dev_args)
        jax.block_until_ready(outs)
        return outs

    def execute_chain(self, k):
        outs = None
        for _ in range(k):
            outs = self._fn(*self._dev_args)
        jax.block_until_ready(outs)
        return outs

    def run(self, in_maps):
        self.prepare(in_maps)
        outs = self.execute()
        return [
            {name: np.asarray(outs[i]).reshape(self.n_cores,
                                               *self.out_avals[i].shape)[c]
             for i, name in enumerate(self.out_names)}
            for c in range(self.n_cores)
        ]


_RUNNERS = {}


def _get_runner(chunk_lens, caps):
    key = (tuple(chunk_lens), tuple(caps))
    if key not in _RUNNERS:
        nc = build_kernel(list(chunk_lens), list(caps))
        _RUNNERS[key] = _Runner(nc, NC)
    return _RUNNERS[key]


def dispatch(hidden_states, top_k_index, top_k_weights,
             chunk_bounds=CHUNK_BOUNDS):
    """Host all-to-all dispatch with pre-tiled DMA layouts."""
    hs = np.asarray(hidden_states, dtype=np.float32)
    idx = np.asarray(top_k_index).astype(np.int64)
    tw = np.asarray(top_k_weights, dtype=np.float32)

    bounds = [0] + list(chunk_bounds)
    chunk_lens = [bounds[j + 1] - bounds[j] for j in range(len(bounds) - 1)]
    nchunks = len(chunk_lens)

    w = np.zeros((E, T), dtype=np.float32)
    tarange = np.arange(T)
    for k in range(KTOP):
        np.add.at(w, (idx[:, k], tarange), tw[:, k])
    routed = np.zeros((E, T), dtype=bool)
    for k in range(KTOP):
        routed[idx[:, k], tarange] = True

    caps = []
    for j in range(nchunks):
        counts = routed[:, bounds[j]:bounds[j + 1]].sum(axis=1)
        caps.append(max(128, int(-(-counts.max() // 128) * 128)))
    C = sum(caps)
    seg_lo = [sum(caps[:jj]) for jj in range(nchunks)]

    hsT_bf = np.ascontiguousarray(hs.T).astype(ml_dtypes.bfloat16)
    blocks = _blocks_of(C, caps)
    hs_el = sum(HC * nb for _, nb in blocks)

    in_maps = []
    for e in range(E):
        cols = np.zeros(C, dtype=np.int64)
        wcg = np.zeros(C, dtype=np.float32)
        sidx = np.zeros(C, dtype=np.int32)
        for j in range(nchunks):
            toks = np.nonzero(routed[e, bounds[j]:bounds[j + 1]])[0] + bounds[j]
            n = len(toks)
            base = seg_lo[j]
            cols[base:base + n] = toks
            wcg[base:base + n] = w[e, toks]
            sidx[base:base + n] = (toks - bounds[j]).astype(np.int32)
            sidx[base + n:base + caps[j]] = chunk_lens[j]  # trash row
        hsg = hsT_bf[:, cols]  # [H, C]
        hsTiled = np.empty((128, hs_el), dtype=ml_dtypes.bfloat16)
        off = 0
        for (pos, nb) in blocks:
            X = np.ascontiguousarray(hsg[:, pos:pos + nb])
            hsTiled[:, off:off + HC * nb] = (
                X.reshape(HC, 128, nb).transpose(1, 0, 2).reshape(128, HC * nb))
            off += HC * nb
        in_maps.append({"hsTiled": hsTiled, "wcg": wcg, "sidx": sidx})
    return in_maps, chunk_lens, caps


def tile_weights(Wg_e, Wu_e, Wd_e):
    wg = np.asarray(Wg_e, dtype=np.float32).astype(ml_dtypes.bfloat16)
    wu = np.asarray(Wu_e, dtype=np.float32).astype(ml_dtypes.bfloat16)
    wd = np.asarray(Wd_e, dtype=np.float32).astype(ml_dtypes.bfloat16)
    wgT = wg.reshape(HC, 128, I).transpose(1, 0, 2).reshape(128, HC * I)
    wuT = wu.reshape(HC, 128, I).transpose(1, 0, 2).reshape(128, HC * I)
    wdT = wd.reshape(IC, 128, H).transpose(1, 0, 2).reshape(128, IC * H)
    return (np.ascontiguousarray(wgT), np.ascontiguousarray(wuT),
            np.ascontiguousarray(wdT))


def assemble(results, chunk_lens):
    full = np.empty((T, H), dtype=np.float32)
    bounds = np.cumsum([0] + list(chunk_lens))
    out_lo = [bounds[j] // NC for j in range(len(chunk_lens))]
    for c in range(NC):
        r = np.asarray(results[c]["out"], dtype=np.float32)
        for j, L in enumerate(chunk_lens):
            sh = L // NC
            full[bounds[j] + c * sh: bounds[j] + (c + 1) * sh, :] = \
                r[out_lo[j]:out_lo[j] + sh]
    return full


def kernel(hidden_states, top_k_index, top_k_weights, Wg, Wu, Wd):
    in_maps, chunk_lens, caps = dispatch(hidden_states, top_k_index,
                                         top_k_weights)
    for e in range(E):
        wgT, wuT, wdT = tile_weights(Wg[e], Wu[e], Wd[e])
        in_maps[e].update(wgT=wgT, wuT=wuT, wdT=wdT)
    runner = _get_runner(chunk_lens, caps)
    results = runner.run(in_maps)
    return assemble(results, chunk_lens)


# revision 10
# speedup vs baseline: 2.3070x; 2.3070x over previous
"""MoE (top-2 of 8 experts, SwiGLU) kernel for 8 TRN2 NeuronCores.

Expert-parallel sparse strategy. Core e receives only the tokens routed to
expert e (host-side all-to-all dispatch, grouped by token-range chunk and
padded to a per-chunk uniform capacity so all 8 cores run one SPMD program):

  phase 1: aT = silu(hs@Wg)*(hs@Wu) for the compact token set (bf16 matmuls,
           fp32 PSUM), blocks of <=512 tokens.
  phase 2: y = (aT @ Wd) * combine_weight, token-major 128-row tiles,
           indirect-scattered into per-chunk DRAM buffers at chunk-local
           token rows (pads land on a trash row).
  combine: per-chunk ReduceScatter(add) across the 8 cores, issued inline
           right after the chunk's last scatter so it overlaps the next
           chunk's compute; the RS writes each core's token shard directly.

DMA layouts are host-pre-tiled so every device DMA is contiguous per
partition (the naive [H, C] column-slice loads cost ~114us in 1KB lines).

Matmul operands are bf16 (fp32 accumulation): rel err vs the fp32 reference
~5e-3, well inside the 2e-2 gate.
"""

import numpy as np
import ml_dtypes

import jax
import concourse.bass as bass
import concourse.tile as tile
from concourse import bacc, mybir
from concourse.bass import ts

E, H, I, T, KTOP = 8, 2048, 1408, 4096, 2
NC = 8
HC, IC = H // 128, I // 128
BF16 = mybir.dt.bfloat16
F32 = mybir.dt.float32

CHUNK_BOUNDS = (2560, 4096)


def _blocks_of(C, caps=None):
    """Phase-1 blocks of <=512 compact slots. (A segment-aligned variant --
    cutting blocks at chunk boundaries so the first chunk's ReduceScatter
    issues earlier -- measured 2.3ms on its one sample; possibly an ambient
    spike, but reverted to the repeatedly-validated layout.)"""
    blocks, pos = [], 0
    while pos < C:
        nb = min(512, C - pos)
        blocks.append((pos, nb))
        pos += nb
    return blocks


def build_kernel(chunk_lens, caps, rs_to_out=False, wsplit=True,
                 silu_fused=True, use_rs=True, defer_zero=True):
    """One SPMD program. chunk_lens[j]: token count of chunk j (mult of NC);
    caps[j]: compact-slot capacity per chunk (mult of 128). Per-core output:
    [T//NC, H] bf16, rows grouped by chunk."""
    nchunks = len(chunk_lens)
    assert sum(chunk_lens) == T
    C = sum(caps)
    assert all(c % 128 == 0 for c in caps)
    assert all(l % NC == 0 for l in chunk_lens)

    nc = bacc.Bacc("TRN2", target_bir_lowering=False, debug=False,
                   num_devices=NC)
    blocks = _blocks_of(C, caps)
    hs_el = sum(HC * nb for _, nb in blocks)
    hsTiled = nc.declare_dram_parameter("hsTiled", [128, hs_el], BF16,
                                        isOutput=False).ap()
    wgT = nc.declare_dram_parameter("wgT", [128, HC * I], BF16, isOutput=False).ap()
    wuT = nc.declare_dram_parameter("wuT", [128, HC * I], BF16, isOutput=False).ap()
    wdT = nc.declare_dram_parameter("wdT", [128, IC * H], BF16, isOutput=False).ap()
    wcg = nc.declare_dram_parameter("wcg", [C], F32, isOutput=False).ap()
    sidx = nc.declare_dram_parameter("sidx", [C], mybir.dt.int32,
                                     isOutput=False).ap()
    out = nc.declare_dram_parameter("out", [T // NC, H], BF16,
                                    isOutput=True).ap()

    silu = mybir.ActivationFunctionType.Silu
    rgroups = [list(range(NC))]
    nct = C // 128
    seg_lo = [sum(caps[:j]) for j in range(nchunks)]
    out_lo = [sum(chunk_lens[:j]) // NC for j in range(nchunks)]

    with tile.TileContext(nc) as tc:
        with (
            tc.tile_pool(name="wpool", bufs=1) as wpool,
            tc.tile_pool(name="hspool", bufs=2) as hspool,
            tc.tile_pool(name="apool", bufs=1) as apool,
            tc.tile_pool(name="stage", bufs=3) as stage,
            tc.tile_pool(name="ypool", bufs=3) as ypool,
            tc.tile_pool(name="pg", bufs=2, space="PSUM") as pg,
            tc.tile_pool(name="pu", bufs=2, space="PSUM") as pu,
            tc.tile_pool(name="py", bufs=2, space="PSUM") as py,
            tc.tile_pool(name="dram", bufs=1, space="DRAM") as dram,
        ):
            wg_sb = wpool.tile([128, HC, I], BF16, tag="wg")
            wu_sb = wpool.tile([128, HC, I], BF16, tag="wu")
            wd_sb = wpool.tile([128, IC, H], BF16, tag="wd")
            wgv = wgT.rearrange("p (c i) -> p c i", c=HC)
            wuv = wuT.rearrange("p (c i) -> p c i", c=HC)
            wdv = wdT.rearrange("p (c j) -> p c j", c=IC)
            if wsplit:
                for c in range(HC):
                    nc.sync.dma_start(out=wg_sb[:, c, :], in_=wgv[:, c, :])
                    nc.sync.dma_start(out=wu_sb[:, c, :], in_=wuv[:, c, :])
                for c in range(IC):
                    nc.sync.dma_start(out=wd_sb[:, c, :], in_=wdv[:, c, :])
            else:
                nc.sync.dma_start(out=wg_sb[:], in_=wgv)
                nc.sync.dma_start(out=wu_sb[:], in_=wuv)
                nc.sync.dma_start(out=wd_sb[:], in_=wdv)
            wcg_sb = wpool.tile([128, nct], F32, tag="wcg")
            nc.sync.dma_start(out=wcg_sb[:], in_=wcg.rearrange("(ct p) -> p ct", p=128))
            sidx_sb = wpool.tile([128, nct], mybir.dt.int32, tag="sidx")
            nc.sync.dma_start(out=sidx_sb[:], in_=sidx.rearrange("(ct p) -> p ct", p=128))
            zsb = wpool.tile([128, H], BF16, tag="zero")
            nc.vector.memset(zsb[:], 0.0)

            pts, rss = [], []
            for j in range(nchunks):
                pt = dram.tile([chunk_lens[j] + 128, H], BF16,
                               name=f"pt{j}", tag=f"pt{j}")
                pts.append(pt)
                if not defer_zero or j == 0:
                    for q in range(chunk_lens[j] // 128):
                        nc.sync.dma_start(out=pt[ts(q, 128), :], in_=zsb[:])
                rss.append(dram.tile([chunk_lens[j] // NC, H], BF16,
                                     name=f"rs{j}", tag=f"rs{j}"))

            def finish_chunk(j):
                sh = chunk_lens[j] // NC
                if use_rs and rs_to_out:
                    nc.gpsimd.collective_compute(
                        "ReduceScatter", mybir.AluOpType.add,
                        replica_groups=rgroups,
                        ins=[pts[j][:chunk_lens[j], :].opt()],
                        outs=[out[out_lo[j]:out_lo[j] + sh, :].opt()])
                elif use_rs:
                    nc.gpsimd.collective_compute(
                        "ReduceScatter", mybir.AluOpType.add,
                        replica_groups=rgroups,
                        ins=[pts[j][:chunk_lens[j], :].opt()],
                        outs=[rss[j][:].opt()])
                    nc.sync.dma_start(out=out[out_lo[j]:out_lo[j] + sh, :],
                                      in_=rss[j][:])
                else:
                    nc.sync.dma_start(out=out[out_lo[j]:out_lo[j] + sh, :],
                                      in_=pts[j][:sh, :])

            hs_off = 0
            zeros_pending = defer_zero
            for (pos, nb) in blocks:
                hs_t = hspool.tile([128, HC, nb], BF16, tag="hst")
                nc.sync.dma_start(
                    out=hs_t[:],
                    in_=hsTiled[:, hs_off:hs_off + HC * nb]
                        .rearrange("p (c t) -> p c t", c=HC))
                hs_off += HC * nb

                if zeros_pending:
                    # deferred pt zeroing: issued after block0's hs load so
                    # these 8+MB of writes stay off the startup DMA window
                    zeros_pending = False
                    for j in range(1, nchunks):
                        for q in range(chunk_lens[j] // 128):
                            nc.sync.dma_start(out=pts[j][ts(q, 128), :],
                                              in_=zsb[:])

                aT = apool.tile([128, IC, nb], BF16, tag="aT")
                for it in range(IC):
                    psg = pg.tile([128, nb], F32, tag="psg")
                    psu = pu.tile([128, nb], F32, tag="psu")
                    for c in range(HC):
                        nc.tensor.matmul(psg[:], lhsT=wg_sb[:, c, ts(it, 128)],
                                         rhs=hs_t[:, c, :],
                                         start=(c == 0), stop=(c == HC - 1))
                    for c in range(HC):
                        nc.tensor.matmul(psu[:], lhsT=wu_sb[:, c, ts(it, 128)],
                                         rhs=hs_t[:, c, :],
                                         start=(c == 0), stop=(c == HC - 1))
                    sil = stage.tile([128, nb], F32, tag="sil")
                    if silu_fused:
                        nc.scalar.activation(out=sil[:], in_=psg[:], func=silu)
                        nc.vector.tensor_mul(aT[:, it, :], sil[:], psu[:])
                    else:
                        nc.scalar.activation(
                            out=sil[:], in_=psg[:],
                            func=mybir.ActivationFunctionType.Sigmoid)
                        nc.vector.tensor_mul(sil[:], sil[:], psg[:])
                        nc.vector.tensor_mul(aT[:, it, :], sil[:], psu[:])

                for ct in range(nb // 128):
                    gct = pos // 128 + ct
                    row = gct * 128
                    j = next(jj for jj in range(nchunks)
                             if seg_lo[jj] <= row < seg_lo[jj] + caps[jj])
                    y_sb = ypool.tile([128, H], BF16, tag="ysb")
                    for hb in range(H // 512):
                        psy = py.tile([128, 512], F32, tag="psy")
                        for c2 in range(IC):
                            nc.tensor.matmul(psy[:],
                                             lhsT=aT[:, c2, ts(ct, 128)],
                                             rhs=wd_sb[:, c2, ts(hb, 512)],
                                             start=(c2 == 0), stop=(c2 == IC - 1))
                        nc.vector.tensor_scalar_mul(
                            y_sb[:, ts(hb, 512)], psy[:],
                            wcg_sb[:, gct:gct + 1])
                    nc.gpsimd.indirect_dma_start(
                        out=pts[j][:],
                        out_offset=bass.IndirectOffsetOnAxis(
                            ap=sidx_sb[:, gct:gct + 1], axis=0),
                        in_=y_sb[:],
                        in_offset=None)
                    if row + 128 == seg_lo[j] + caps[j]:
                        finish_chunk(j)

    nc.compile()
    return nc


class _Runner:
    """Compile once, execute many; device-resident staged inputs."""

    def __init__(self, nc, n_cores):
        from concourse import bass2jax
        from jax.experimental.shard_map import shard_map
        from jax.sharding import Mesh, PartitionSpec

        bass2jax.install_neuronx_cc_hook()
        partition_name = (nc.partition_id_tensor.name
                          if nc.partition_id_tensor else None)

        in_names, out_names, out_avals, zero_outs = [], [], [], []
        for alloc in nc.m.functions[0].allocations:
            if not isinstance(alloc, mybir.MemoryLocationSet):
                continue
            name = alloc.memorylocations[0].name
            if alloc.kind == "ExternalInput":
                if name != partition_name:
                    in_names.append(name)
            elif alloc.kind == "ExternalOutput":
                shape = tuple(alloc.tensor_shape)
                dtype = mybir.dt.np(alloc.dtype)
                out_names.append(name)
                out_avals.append(jax.core.ShapedArray(shape, dtype))
                zero_outs.append(np.zeros(shape, dtype))
        self.n_params = len(in_names)
        self.param_names = list(in_names)
        self.out_names = out_names
        self.out_avals = out_avals
        self.n_cores = n_cores
        all_names = in_names + out_names
        if partition_name is not None:
            all_names.append(partition_name)

        def _body(*args):
            operands = list(args)
            if partition_name is not None:
                operands.append(bass2jax.partition_id_tensor())
            outs = bass2jax._bass_exec_p.bind(
                *operands,
                out_avals=tuple(out_avals),
                in_names=tuple(all_names),
                out_names=tuple(out_names),
                lowering_input_output_aliases=(),
                sim_require_finite=True,
                sim_require_nnan=True,
                nc=nc,
            )
            return tuple(outs)

        devices = jax.devices()[:n_cores]
        assert len(devices) == n_cores
        mesh = Mesh(np.asarray(devices), ("core",))
        n_ops = self.n_params + len(out_names)
        self._mesh = mesh
        self._fn = jax.jit(
            shard_map(_body, mesh=mesh,
                      in_specs=(PartitionSpec("core"),) * n_ops,
                      out_specs=(PartitionSpec("core"),) * len(out_names),
                      check_rep=False),
            keep_unused=True)
        self._zeros = [
            np.zeros((n_cores * z.shape[0], *z.shape[1:]), z.dtype)
            for z in zero_outs
        ]
        self._dev_args = None

    def prepare(self, in_maps):
        from jax.sharding import NamedSharding, PartitionSpec
        sh = NamedSharding(self._mesh, PartitionSpec("core"))
        concat = [
            np.concatenate([np.asarray(in_maps[c][name])
                            for c in range(self.n_cores)], axis=0)
            for name in self.param_names
        ]
        self._dev_args = [jax.device_put(a, sh) for a in concat + self._zeros]

    def execute(self):
        outs = self._fn(*self._dev_args)
        jax.block_until_ready(outs)
        return outs

    def execute_chain(self, k):
        outs = None
        for _ in range(k):
            outs = self._fn(*self._dev_args)
        jax.block_until_ready(outs)
        return outs

    def run(self, in_maps):
        self.prepare(in_maps)
        outs = self.execute()
        return [
            {name: np.asarray(outs[i]).reshape(self.n_cores,
                                               *self.out_avals[i].shape)[c]
             for i, name in enumerate(self.out_names)}
            for c in range(self.n_cores)
        ]


_RUNNERS = {}


def _get_runner(chunk_lens, caps):
    key = (tuple(chunk_lens), tuple(caps))
    if key not in _RUNNERS:
        nc = build_kernel(list(chunk_lens), list(caps))
        _RUNNERS[key] = _Runner(nc, NC)
    return _RUNNERS[key]


def dispatch(hidden_states, top_k_index, top_k_weights,
             chunk_bounds=CHUNK_BOUNDS):
    """Host all-to-all dispatch with pre-tiled DMA layouts."""
    hs = np.asarray(hidden_states, dtype=np.float32)
    idx = np.asarray(top_k_index).astype(np.int64)
    tw = np.asarray(top_k_weights, dtype=np.float32)

    bounds = [0] + list(chunk_bounds)
    chunk_lens = [bounds[j + 1] - bounds[j] for j in range(len(bounds) - 1)]
    nchunks = len(chunk_lens)

    w = np.zeros((E, T), dtype=np.float32)
    tarange = np.arange(T)
    for k in range(KTOP):
        np.add.at(w, (idx[:, k], tarange), tw[:, k])
    routed = np.zeros((E, T), dtype=bool)
    for k in range(KTOP):
        routed[idx[:, k], tarange] = True

    caps = []
    for j in range(nchunks):
        counts = routed[:, bounds[j]:bounds[j + 1]].sum(axis=1)
        caps.append(max(128, int(-(-counts.max() // 128) * 128)))
    C = sum(caps)
    seg_lo = [sum(caps[:jj]) for jj in range(nchunks)]

    hsT_bf = np.ascontiguousarray(hs.T).astype(ml_dtypes.bfloat16)
    blocks = _blocks_of(C, caps)
    hs_el = sum(HC * nb for _, nb in blocks)

    in_maps = []
    for e in range(E):
        cols = np.zeros(C, dtype=np.int64)
        wcg = np.zeros(C, dtype=np.float32)
        sidx = np.zeros(C, dtype=np.int32)
        for j in range(nchunks):
            toks = np.nonzero(routed[e, bounds[j]:bounds[j + 1]])[0] + bounds[j]
            n = len(toks)
            base = seg_lo[j]
            cols[base:base + n] = toks
            wcg[base:base + n] = w[e, toks]
            sidx[base:base + n] = (toks - bounds[j]).astype(np.int32)
            sidx[base + n:base + caps[j]] = chunk_lens[j]  # trash row
        hsg = hsT_bf[:, cols]  # [H, C]
        hsTiled = np.empty((128, hs_el), dtype=ml_dtypes.bfloat16)
        off = 0
        for (pos, nb) in blocks:
            X = np.ascontiguousarray(hsg[:, pos:pos + nb])
            hsTiled[:, off:off + HC * nb] = (
                X.reshape(HC, 128, nb).transpose(1, 0, 2).reshape(128, HC * nb))
            off += HC * nb
        in_maps.append({"hsTiled": hsTiled, "wcg": wcg, "sidx": sidx})
    return in_maps, chunk_lens, caps


def tile_weights(Wg_e, Wu_e, Wd_e):
    wg = np.asarray(Wg_e, dtype=np.float32).astype(ml_dtypes.bfloat16)
    wu = np.asarray(Wu_e, dtype=np.float32).astype(ml_dtypes.bfloat16)
    wd = np.asarray(Wd_e, dtype=np.float32).astype(ml_dtypes.bfloat16)
    wgT = wg.reshape(HC, 128, I).transpose(1, 0, 2).reshape(128, HC * I)
    wuT = wu.reshape(HC, 128, I).transpose(1, 0, 2).reshape(128, HC * I)
    wdT = wd.reshape(IC, 128, H).transpose(1, 0, 2).reshape(128, IC * H)
    return (np.ascontiguousarray(wgT), np.ascontiguousarray(wuT),
            np.ascontiguousarray(wdT))


def assemble(results, chunk_lens):
    full = np.empty((T, H), dtype=np.float32)
    bounds = np.cumsum([0] + list(chunk_lens))
    out_lo = [bounds[j] // NC for j in range(len(chunk_lens))]
    for c in range(NC):
        r = np.asarray(results[c]["out"], dtype=np.float32)
        for j, L in enumerate(chunk_lens):
            sh = L // NC
            full[bounds[j] + c * sh: bounds[j] + (c + 1) * sh, :] = \
                r[out_lo[j]:out_lo[j] + sh]
    return full


def kernel(hidden_states, top_k_index, top_k_weights, Wg, Wu, Wd):
    in_maps, chunk_lens, caps = dispatch(hidden_states, top_k_index,
                                         top_k_weights)
    for e in range(E):
        wgT, wuT, wdT = tile_weights(Wg[e], Wu[e], Wd[e])
        in_maps[e].update(wgT=wgT, wuT=wuT, wdT=wdT)
    runner = _get_runner(chunk_lens, caps)
    results = runner.run(in_maps)
    return assemble(results, chunk_lens)


# revision 11
# speedup vs baseline: 3.1612x; 1.3703x over previous
"""MoE (top-2 of 8 experts, SwiGLU) kernel for 8 TRN2 NeuronCores.

Expert-parallel sparse strategy. Core e receives only the tokens routed to
expert e (host-side all-to-all dispatch, grouped by token-range chunk and
padded to a per-chunk uniform capacity so all 8 cores run one SPMD program):

  phase 1: aT = silu(hs@Wg)*(hs@Wu) for the compact token set (bf16 matmuls,
           fp32 PSUM), blocks of <=512 tokens.
  phase 2: y = (aT @ Wd) * combine_weight, token-major 128-row tiles,
           indirect-scattered into per-chunk DRAM buffers at chunk-local
           token rows (pads land on a trash row).
  combine: per-chunk ReduceScatter(add) across the 8 cores, issued inline
           right after the chunk's last scatter so it overlaps the next
           chunk's compute; the RS writes each core's token shard directly.

DMA layouts are host-pre-tiled so every device DMA is contiguous per
partition (the naive [H, C] column-slice loads cost ~114us in 1KB lines).

Matmul operands are bf16 (fp32 accumulation): rel err vs the fp32 reference
~5e-3, well inside the 2e-2 gate.
"""

import numpy as np
import ml_dtypes

import jax
import concourse.bass as bass
import concourse.tile as tile
from concourse import bacc, mybir
from concourse.bass import ts

E, H, I, T, KTOP = 8, 2048, 1408, 4096, 2
NC = 8
HC, IC = H // 128, I // 128
BF16 = mybir.dt.bfloat16
F32 = mybir.dt.float32

CHUNK_BOUNDS = (2560, 4096)


def _blocks_of(C, caps=None):
    """Phase-1 blocks of <=512 compact slots, cut at chunk-segment
    boundaries: each chunk's last scatter (hence its ReduceScatter) then
    issues with the entire next segment's compute still pending to hide
    behind. Within a segment, blocks are balanced 128-multiples (640 ->
    384+256, not 512+128) so no block is too small for the PE pipeline."""
    if caps is None:
        caps = [C]
    blocks, pos = [], 0
    for cap in caps:
        cap = int(cap)
        k = -(-cap // 512)  # blocks in this segment
        base, extra = divmod(cap // 128, k)
        for b in range(k):
            nb = (base + (1 if b < extra else 0)) * 128
            blocks.append((pos, nb))
            pos += nb
    return blocks


def build_kernel(chunk_lens, caps, rs_to_out=False, wsplit=True,
                 silu_fused=True, use_rs=True, defer_zero=True):
    """One SPMD program. chunk_lens[j]: token count of chunk j (mult of NC);
    caps[j]: compact-slot capacity per chunk (mult of 128). Per-core output:
    [T//NC, H] bf16, rows grouped by chunk."""
    nchunks = len(chunk_lens)
    assert sum(chunk_lens) == T
    C = sum(caps)
    assert all(c % 128 == 0 for c in caps)
    assert all(l % NC == 0 for l in chunk_lens)

    nc = bacc.Bacc("TRN2", target_bir_lowering=False, debug=False,
                   num_devices=NC)
    blocks = _blocks_of(C, caps)
    hs_el = sum(HC * nb for _, nb in blocks)
    hsTiled = nc.declare_dram_parameter("hsTiled", [128, hs_el], BF16,
                                        isOutput=False).ap()
    wgT = nc.declare_dram_parameter("wgT", [128, HC * I], BF16, isOutput=False).ap()
    wuT = nc.declare_dram_parameter("wuT", [128, HC * I], BF16, isOutput=False).ap()
    wdT = nc.declare_dram_parameter("wdT", [128, IC * H], BF16, isOutput=False).ap()
    wcg = nc.declare_dram_parameter("wcg", [C], F32, isOutput=False).ap()
    sidx = nc.declare_dram_parameter("sidx", [C], mybir.dt.int32,
                                     isOutput=False).ap()
    out = nc.declare_dram_parameter("out", [T // NC, H], BF16,
                                    isOutput=True).ap()

    silu = mybir.ActivationFunctionType.Silu
    rgroups = [list(range(NC))]
    nct = C // 128
    seg_lo = [sum(caps[:j]) for j in range(nchunks)]
    out_lo = [sum(chunk_lens[:j]) // NC for j in range(nchunks)]

    with tile.TileContext(nc) as tc:
        with (
            tc.tile_pool(name="wpool", bufs=1) as wpool,
            tc.tile_pool(name="hspool", bufs=2) as hspool,
            tc.tile_pool(name="apool", bufs=1) as apool,
            tc.tile_pool(name="stage", bufs=3) as stage,
            tc.tile_pool(name="ypool", bufs=3) as ypool,
            tc.tile_pool(name="pg", bufs=2, space="PSUM") as pg,
            tc.tile_pool(name="pu", bufs=2, space="PSUM") as pu,
            tc.tile_pool(name="py", bufs=2, space="PSUM") as py,
            tc.tile_pool(name="dram", bufs=1, space="DRAM") as dram,
        ):
            wg_sb = wpool.tile([128, HC, I], BF16, tag="wg")
            wu_sb = wpool.tile([128, HC, I], BF16, tag="wu")
            wd_sb = wpool.tile([128, IC, H], BF16, tag="wd")
            wgv = wgT.rearrange("p (c i) -> p c i", c=HC)
            wuv = wuT.rearrange("p (c i) -> p c i", c=HC)
            wdv = wdT.rearrange("p (c j) -> p c j", c=IC)
            if wsplit:
                for c in range(HC):
                    nc.sync.dma_start(out=wg_sb[:, c, :], in_=wgv[:, c, :])
                    nc.sync.dma_start(out=wu_sb[:, c, :], in_=wuv[:, c, :])
                for c in range(IC):
                    nc.sync.dma_start(out=wd_sb[:, c, :], in_=wdv[:, c, :])
            else:
                nc.sync.dma_start(out=wg_sb[:], in_=wgv)
                nc.sync.dma_start(out=wu_sb[:], in_=wuv)
                nc.sync.dma_start(out=wd_sb[:], in_=wdv)
            wcg_sb = wpool.tile([128, nct], F32, tag="wcg")
            nc.sync.dma_start(out=wcg_sb[:], in_=wcg.rearrange("(ct p) -> p ct", p=128))
            sidx_sb = wpool.tile([128, nct], mybir.dt.int32, tag="sidx")
            nc.sync.dma_start(out=sidx_sb[:], in_=sidx.rearrange("(ct p) -> p ct", p=128))
            zsb = wpool.tile([128, H], BF16, tag="zero")
            nc.vector.memset(zsb[:], 0.0)

            pts, rss = [], []
            for j in range(nchunks):
                pt = dram.tile([chunk_lens[j] + 128, H], BF16,
                               name=f"pt{j}", tag=f"pt{j}")
                pts.append(pt)
                if not defer_zero or j == 0:
                    for q in range(chunk_lens[j] // 128):
                        nc.sync.dma_start(out=pt[ts(q, 128), :], in_=zsb[:])
                rss.append(dram.tile([chunk_lens[j] // NC, H], BF16,
                                     name=f"rs{j}", tag=f"rs{j}"))

            def finish_chunk(j):
                sh = chunk_lens[j] // NC
                if use_rs and rs_to_out:
                    nc.gpsimd.collective_compute(
                        "ReduceScatter", mybir.AluOpType.add,
                        replica_groups=rgroups,
                        ins=[pts[j][:chunk_lens[j], :].opt()],
                        outs=[out[out_lo[j]:out_lo[j] + sh, :].opt()])
                elif use_rs:
                    nc.gpsimd.collective_compute(
                        "ReduceScatter", mybir.AluOpType.add,
                        replica_groups=rgroups,
                        ins=[pts[j][:chunk_lens[j], :].opt()],
                        outs=[rss[j][:].opt()])
                    nc.sync.dma_start(out=out[out_lo[j]:out_lo[j] + sh, :],
                                      in_=rss[j][:])
                else:
                    nc.sync.dma_start(out=out[out_lo[j]:out_lo[j] + sh, :],
                                      in_=pts[j][:sh, :])

            hs_off = 0
            zeros_pending = defer_zero
            for (pos, nb) in blocks:
                hs_t = hspool.tile([128, HC, nb], BF16, tag="hst")
                nc.sync.dma_start(
                    out=hs_t[:],
                    in_=hsTiled[:, hs_off:hs_off + HC * nb]
                        .rearrange("p (c t) -> p c t", c=HC))
                hs_off += HC * nb

                if zeros_pending:
                    # deferred pt zeroing: issued after block0's hs load so
                    # these 8+MB of writes stay off the startup DMA window
                    zeros_pending = False
                    for j in range(1, nchunks):
                        for q in range(chunk_lens[j] // 128):
                            nc.sync.dma_start(out=pts[j][ts(q, 128), :],
                                              in_=zsb[:])

                aT = apool.tile([128, IC, nb], BF16, tag="aT")
                for it in range(IC):
                    psg = pg.tile([128, nb], F32, tag="psg")
                    psu = pu.tile([128, nb], F32, tag="psu")
                    for c in range(HC):
                        nc.tensor.matmul(psg[:], lhsT=wg_sb[:, c, ts(it, 128)],
                                         rhs=hs_t[:, c, :],
                                         start=(c == 0), stop=(c == HC - 1))
                    for c in range(HC):
                        nc.tensor.matmul(psu[:], lhsT=wu_sb[:, c, ts(it, 128)],
                                         rhs=hs_t[:, c, :],
                                         start=(c == 0), stop=(c == HC - 1))
                    sil = stage.tile([128, nb], F32, tag="sil")
                    if silu_fused:
                        nc.scalar.activation(out=sil[:], in_=psg[:], func=silu)
                        nc.vector.tensor_mul(aT[:, it, :], sil[:], psu[:])
                    else:
                        nc.scalar.activation(
                            out=sil[:], in_=psg[:],
                            func=mybir.ActivationFunctionType.Sigmoid)
                        nc.vector.tensor_mul(sil[:], sil[:], psg[:])
                        nc.vector.tensor_mul(aT[:, it, :], sil[:], psu[:])

                for ct in range(nb // 128):
                    gct = pos // 128 + ct
                    row = gct * 128
                    j = next(jj for jj in range(nchunks)
                             if seg_lo[jj] <= row < seg_lo[jj] + caps[jj])
                    y_sb = ypool.tile([128, H], BF16, tag="ysb")
                    for hb in range(H // 512):
                        psy = py.tile([128, 512], F32, tag="psy")
                        for c2 in range(IC):
                            nc.tensor.matmul(psy[:],
                                             lhsT=aT[:, c2, ts(ct, 128)],
                                             rhs=wd_sb[:, c2, ts(hb, 512)],
                                             start=(c2 == 0), stop=(c2 == IC - 1))
                        nc.vector.tensor_scalar_mul(
                            y_sb[:, ts(hb, 512)], psy[:],
                            wcg_sb[:, gct:gct + 1])
                    nc.gpsimd.indirect_dma_start(
                        out=pts[j][:],
                        out_offset=bass.IndirectOffsetOnAxis(
                            ap=sidx_sb[:, gct:gct + 1], axis=0),
                        in_=y_sb[:],
                        in_offset=None)
                    if row + 128 == seg_lo[j] + caps[j]:
                        finish_chunk(j)

    nc.compile()
    return nc


class _Runner:
    """Compile once, execute many; device-resident staged inputs."""

    def __init__(self, nc, n_cores):
        from concourse import bass2jax
        from jax.experimental.shard_map import shard_map
        from jax.sharding import Mesh, PartitionSpec

        bass2jax.install_neuronx_cc_hook()
        partition_name = (nc.partition_id_tensor.name
                          if nc.partition_id_tensor else None)

        in_names, out_names, out_avals, zero_outs = [], [], [], []
        for alloc in nc.m.functions[0].allocations:
            if not isinstance(alloc, mybir.MemoryLocationSet):
                continue
            name = alloc.memorylocations[0].name
            if alloc.kind == "ExternalInput":
                if name != partition_name:
                    in_names.append(name)
            elif alloc.kind == "ExternalOutput":
                shape = tuple(alloc.tensor_shape)
                dtype = mybir.dt.np(alloc.dtype)
                out_names.append(name)
                out_avals.append(jax.core.ShapedArray(shape, dtype))
                zero_outs.append(np.zeros(shape, dtype))
        self.n_params = len(in_names)
        self.param_names = list(in_names)
        self.out_names = out_names
        self.out_avals = out_avals
        self.n_cores = n_cores
        all_names = in_names + out_names
        if partition_name is not None:
            all_names.append(partition_name)

        def _body(*args):
            operands = list(args)
            if partition_name is not None:
                operands.append(bass2jax.partition_id_tensor())
            outs = bass2jax._bass_exec_p.bind(
                *operands,
                out_avals=tuple(out_avals),
                in_names=tuple(all_names),
                out_names=tuple(out_names),
                lowering_input_output_aliases=(),
                sim_require_finite=True,
                sim_require_nnan=True,
                nc=nc,
            )
            return tuple(outs)

        devices = jax.devices()[:n_cores]
        assert len(devices) == n_cores
        mesh = Mesh(np.asarray(devices), ("core",))
        n_ops = self.n_params + len(out_names)
        self._mesh = mesh
        self._fn = jax.jit(
            shard_map(_body, mesh=mesh,
                      in_specs=(PartitionSpec("core"),) * n_ops,
                      out_specs=(PartitionSpec("core"),) * len(out_names),
                      check_rep=False),
            keep_unused=True)
        self._zeros = [
            np.zeros((n_cores * z.shape[0], *z.shape[1:]), z.dtype)
            for z in zero_outs
        ]
        self._dev_args = None

    def prepare(self, in_maps):
        from jax.sharding import NamedSharding, PartitionSpec
        sh = NamedSharding(self._mesh, PartitionSpec("core"))
        concat = [
            np.concatenate([np.asarray(in_maps[c][name])
                            for c in range(self.n_cores)], axis=0)
            for name in self.param_names
        ]
        self._dev_args = [jax.device_put(a, sh) for a in concat + self._zeros]

    def execute(self):
        outs = self._fn(*self._dev_args)
        jax.block_until_ready(outs)
        return outs

    def execute_chain(self, k):
        outs = None
        for _ in range(k):
            outs = self._fn(*self._dev_args)
        jax.block_until_ready(outs)
        return outs

    def run(self, in_maps):
        self.prepare(in_maps)
        outs = self.execute()
        return [
            {name: np.asarray(outs[i]).reshape(self.n_cores,
                                               *self.out_avals[i].shape)[c]
             for i, name in enumerate(self.out_names)}
            for c in range(self.n_cores)
        ]


_RUNNERS = {}


def _get_runner(chunk_lens, caps):
    key = (tuple(chunk_lens), tuple(caps))
    if key not in _RUNNERS:
        nc = build_kernel(list(chunk_lens), list(caps))
        _RUNNERS[key] = _Runner(nc, NC)
    return _RUNNERS[key]


def dispatch(hidden_states, top_k_index, top_k_weights,
             chunk_bounds=CHUNK_BOUNDS):
    """Host all-to-all dispatch with pre-tiled DMA layouts."""
    hs = np.asarray(hidden_states, dtype=np.float32)
    idx = np.asarray(top_k_index).astype(np.int64)
    tw = np.asarray(top_k_weights, dtype=np.float32)

    bounds = [0] + list(chunk_bounds)
    chunk_lens = [bounds[j + 1] - bounds[j] for j in range(len(bounds) - 1)]
    nchunks = len(chunk_lens)

    w = np.zeros((E, T), dtype=np.float32)
    tarange = np.arange(T)
    for k in range(KTOP):
        np.add.at(w, (idx[:, k], tarange), tw[:, k])
    routed = np.zeros((E, T), dtype=bool)
    for k in range(KTOP):
        routed[idx[:, k], tarange] = True

    caps = []
    for j in range(nchunks):
        counts = routed[:, bounds[j]:bounds[j + 1]].sum(axis=1)
        caps.append(max(128, int(-(-counts.max() // 128) * 128)))
    C = sum(caps)
    seg_lo = [sum(caps[:jj]) for jj in range(nchunks)]

    hsT_bf = np.ascontiguousarray(hs.T).astype(ml_dtypes.bfloat16)
    blocks = _blocks_of(C, caps)
    hs_el = sum(HC * nb for _, nb in blocks)

    in_maps = []
    for e in range(E):
        cols = np.zeros(C, dtype=np.int64)
        wcg = np.zeros(C, dtype=np.float32)
        sidx = np.zeros(C, dtype=np.int32)
        for j in range(nchunks):
            toks = np.nonzero(routed[e, bounds[j]:bounds[j + 1]])[0] + bounds[j]
            n = len(toks)
            base = seg_lo[j]
            cols[base:base + n] = toks
            wcg[base:base + n] = w[e, toks]
            sidx[base:base + n] = (toks - bounds[j]).astype(np.int32)
            sidx[base + n:base + caps[j]] = chunk_lens[j]  # trash row
        hsg = hsT_bf[:, cols]  # [H, C]
        hsTiled = np.empty((128, hs_el), dtype=ml_dtypes.bfloat16)
        off = 0
        for (pos, nb) in blocks:
            X = np.ascontiguousarray(hsg[:, pos:pos + nb])
            hsTiled[:, off:off + HC * nb] = (
                X.reshape(HC, 128, nb).transpose(1, 0, 2).reshape(128, HC * nb))
            off += HC * nb
        in_maps.append({"hsTiled": hsTiled, "wcg": wcg, "sidx": sidx})
    return in_maps, chunk_lens, caps


def tile_weights(Wg_e, Wu_e, Wd_e):
    wg = np.asarray(Wg_e, dtype=np.float32).astype(ml_dtypes.bfloat16)
    wu = np.asarray(Wu_e, dtype=np.float32).astype(ml_dtypes.bfloat16)
    wd = np.asarray(Wd_e, dtype=np.float32).astype(ml_dtypes.bfloat16)
    wgT = wg.reshape(HC, 128, I).transpose(1, 0, 2).reshape(128, HC * I)
    wuT = wu.reshape(HC, 128, I).transpose(1, 0, 2).reshape(128, HC * I)
    wdT = wd.reshape(IC, 128, H).transpose(1, 0, 2).reshape(128, IC * H)
    return (np.ascontiguousarray(wgT), np.ascontiguousarray(wuT),
            np.ascontiguousarray(wdT))


def assemble(results, chunk_lens):
    full = np.empty((T, H), dtype=np.float32)
    bounds = np.cumsum([0] + list(chunk_lens))
    out_lo = [bounds[j] // NC for j in range(len(chunk_lens))]
    for c in range(NC):
        r = np.asarray(results[c]["out"], dtype=np.float32)
        for j, L in enumerate(chunk_lens):
            sh = L // NC
            full[bounds[j] + c * sh: bounds[j] + (c + 1) * sh, :] = \
                r[out_lo[j]:out_lo[j] + sh]
    return full


def kernel(hidden_states, top_k_index, top_k_weights, Wg, Wu, Wd):
    in_maps, chunk_lens, caps = dispatch(hidden_states, top_k_index,
                                         top_k_weights)
    for e in range(E):
        wgT, wuT, wdT = tile_weights(Wg[e], Wu[e], Wd[e])
        in_maps[e].update(wgT=wgT, wuT=wuT, wdT=wdT)
    runner = _get_runner(chunk_lens, caps)
    results = runner.run(in_maps)
    return assemble(results, chunk_lens)
